# revision 11
# baseline (speedup 1.0000x reference)
"""Trainium2 Bass kernel for nn_Network_75084618269290 (dense_cnn).

Network: 6x conv1d(SAME, K=9) + BN + ReLU stack:
  [B=8, 257, 2000] -> conv(257->512) -> 4x conv(512->512) -> conv(512->257)
Weights are clamped to [-1, 1] (done host-side), BN is inference-mode
(folded host-side into per-channel scale/bias).

Sharding: data-parallel over batch B=8 across the 8 NeuronCores -- one
sample per core, zero collectives. Each conv is computed as 1d-conv =
sum over (ci_block, tap) of 128-contraction matmuls accumulated in PSUM,
with a fused scale+bias+ReLU PSUM->SBUF eviction on the scalar engine.
Matmuls run in float32r (full PE rate at N>=256, ~1e-4 rel err).

Self-contained: hardcodes all shapes; host-side prep = clip + transpose +
BN fold + per-sample sharding.
"""
import sys

for _p in ('/opt/trn_rl_repo',):
    if _p not in sys.path:
        sys.path.insert(0, _p)

import numpy as np

import concourse.bass as bass  # noqa: F401  (bass types used via bacc)
from concourse import bacc
import concourse.tile as tile
import concourse.mybir as mybir
from concourse.bass_utils import run_bass_kernel_spmd

# ---- problem constants (hardcoded per contract) ----
B = 8
INP = 257
C = 512
D = 4
K = 9
T = 2000
PAD = 4
TT = 500              # t-tile (PSUM bank holds 512 fp32)
NT = T // TT          # 4
TP = T + 2 * PAD      # 2008 padded time axis
NCI = C // 128        # 4
NCO = C // 128        # 4
BN_EPS = 1e-5

F32R = mybir.dt.float32r
F32 = mybir.dt.float32

_NC_CACHE = {}


def build_nc():
    """Build + compile the per-core Bass program (SPMD: same NEFF on all 8)."""
    if 'nc' in _NC_CACHE:
        return _NC_CACHE['nc']

    nc = bacc.Bacc(None, target_bir_lowering=False)

    # ---- DRAM I/O ----
    x_main = nc.dram_tensor("x_main", [256, T], F32R, kind="ExternalInput")
    xk9 = nc.dram_tensor("xk9", [K, TP], F32R, kind="ExternalInput")
    # weight layouts are pre-tiled host-side so every per-chunk DMA is
    # partition-contiguous (one long line per partition -> few descriptors)
    w_in_a = nc.dram_tensor("w_in_a", [NCO, 128, K, 2, 128], F32R, kind="ExternalInput")
    w_in_b = nc.dram_tensor("w_in_b", [128, C], F32R, kind="ExternalInput")
    w_mid = nc.dram_tensor("w_mid", [D, NCO, 128, K, NCI, 128], F32R, kind="ExternalInput")
    w_out_a = nc.dram_tensor("w_out_a", [2, 128, K, NCI, 128], F32R, kind="ExternalInput")
    w_out_b = nc.dram_tensor("w_out_b", [128, K, NCI, 1], F32R, kind="ExternalInput")
    # packed BN table: per-partition columns =
    # [sc_in(4), bi_in(4), sc_mid(16), bi_mid(16), sc_out(2), bi_out(2),
    #  sc_out_b(1: row0 only), bi_out_b(1: row0 only)]
    sbtab = nc.dram_tensor("sbtab", [128, 46], F32, kind="ExternalInput")
    y = nc.dram_tensor("y", [INP, T], F32, kind="ExternalOutput")

    Relu = mybir.ActivationFunctionType.Relu

    with tile.TileContext(nc) as tc:
        with tc.tile_pool(name="xin", bufs=1) as xin_p, \
             tc.tile_pool(name="act", bufs=1) as act_p, \
             tc.tile_pool(name="win", bufs=1) as win_p, \
             tc.tile_pool(name="wmid", bufs=2) as wmid_p, \
             tc.tile_pool(name="sb", bufs=1) as sb_p, \
             tc.tile_pool(name="ps", bufs=6, space="PSUM") as ps_p, \
             tc.tile_pool(name="psb", bufs=2, space="PSUM") as psb_p, \
             tc.tile_pool(name="out", bufs=4) as out_p, \
             tc.tile_pool(name="outb", bufs=2) as outb_p:

            # PE pre-warm: dummy matmuls on zeroed scratch keep the PE busy
            # during the initial DMA so HAM un-throttles (1.2->2.4GHz) before
            # the first real matmul group. Scratch memset goes to the idle
            # GpSimd engine so the warm matmuls start as early as possible.
            warm = xin_p.tile([128, 512], F32R, tag="warm")
            nc.gpsimd.memset(warm.bitcast(F32), 0.0)
            warm_ps = ps_p.tile([128, TT], F32, tag="ps")
            for _ in range(12):
                nc.tensor.matmul(warm_ps[:], warm[:, 0:128], warm[:, 0:TT],
                                 start=True, stop=True)

            # ---- input buffer: memset only the pad columns (critical path) ----
            xin = xin_p.tile([128, 2, TP], F32R, tag="xin")
            nc.vector.memset(xin[:, :, 0:PAD].bitcast(F32), 0.0)
            nc.vector.memset(xin[:, :, PAD + T:TP].bitcast(F32), 0.0)
            # split the x DMA so the first matmul group (t=0, cols < 512) can
            # start after the first chunk lands
            x_r = x_main.rearrange("(b p) t -> p b t", p=128)
            XSPL = 504
            # sync HW queue drains FIFO: put exactly what the first 9 matmuls
            # need up front (x ci-block 0 cols<512, wi chunk 0 ci-half 0),
            # then the rest in need order.
            nc.sync.dma_start(out=xin[:, 0:1, PAD:PAD + XSPL], in_=x_r[:, 0:1, 0:XSPL])
            wi0 = win_p.tile([128, K, 2, 128], F32R, tag="wi0")
            nc.sync.dma_start(out=wi0[:, :, 0:1, :], in_=w_in_a[0, :, :, 0:1, :])
            nc.sync.dma_start(out=xin[:, 1:2, PAD:PAD + XSPL], in_=x_r[:, 1:2, 0:XSPL])
            nc.sync.dma_start(out=wi0[:, :, 1:2, :], in_=w_in_a[0, :, :, 1:2, :])
            # xk9/wib padded to full 128-row contraction (zero rows 9..127) so
            # the channel-256 matmul uses all row groups and pipelines cleanly
            xk9t = xin_p.tile([128, TP], F32R, tag="xk9")
            nc.vector.memset(xk9t.bitcast(F32), 0.0)
            nc.sync.dma_start(out=xk9t[0:K, :], in_=xk9[:])
            wib = win_p.tile([128, C], F32R, tag="wib")
            nc.sync.dma_start(out=wib[:], in_=w_in_b[:])
            nc.sync.dma_start(out=xin[:, :, PAD + XSPL:PAD + T], in_=x_r[:, :, XSPL:T])

            wi_t = [wi0]
            for co in range(1, NCO):
                wic = win_p.tile([128, K, 2, 128], F32R, tag=f"wi{co}")
                nc.sync.dma_start(out=wic[:], in_=w_in_a[co])
                wi_t.append(wic)

            a0 = act_p.tile([128, NCI, TP], F32R, tag="a0")
            a1 = act_p.tile([128, NCI, TP], F32R, tag="a1")
            nc.vector.memset(a0[:, :, 0:PAD].bitcast(F32), 0.0)
            nc.vector.memset(a0[:, :, PAD + T:TP].bitcast(F32), 0.0)
            nc.vector.memset(a1[:, :, 0:PAD].bitcast(F32), 0.0)
            nc.vector.memset(a1[:, :, PAD + T:TP].bitcast(F32), 0.0)
            abuf = [a0, a1]

            tab = sb_p.tile([128, 46], F32, tag="tab")
            nc.scalar.dma_start(out=tab[:], in_=sbtab[:])
            sci = tab[:, 0:4].rearrange("p (b o) -> p b o", o=1)
            bii = tab[:, 4:8].rearrange("p (b o) -> p b o", o=1)
            scm = tab[:, 8:24].rearrange("p (l b o) -> p l b o", l=D, o=1)
            bim = tab[:, 24:40].rearrange("p (l b o) -> p l b o", l=D, o=1)
            sco = tab[:, 40:42].rearrange("p (b o) -> p b o", o=1)
            bio = tab[:, 42:44].rearrange("p (b o) -> p b o", o=1)
            scob = tab[0:1, 44:45]
            biob = tab[0:1, 45:46]

            # ---- layer 0: in conv (257 -> 512), writes a0 ----
            for co in range(NCO):
                for t in range(NT):
                    ps = ps_p.tile([128, TT], F32, tag="ps")
                    n = 0
                    nmm = 2 * K + 1
                    for ci in range(2):
                        for k in range(K):
                            nc.tensor.matmul(
                                ps[:],
                                wi_t[co][:, k, ci, :],
                                xin[:, ci, t * TT + k: t * TT + k + TT],
                                start=(n == 0), stop=(n == nmm - 1))
                            n += 1
                    # channel 256 via K=9 contraction on host-im2col'd row
                    nc.tensor.matmul(
                        ps[:],
                        wib[:, co * 128:(co + 1) * 128],
                        xk9t[:, t * TT: t * TT + TT],
                        start=False, stop=True)
                    nc.scalar.activation(
                        out=a0[:, co, PAD + t * TT: PAD + (t + 1) * TT],
                        in_=ps[:], func=Relu,
                        bias=bii[:, co, :], scale=sci[:, co, :])

            # ---- layers 1..4: mid convs (512 -> 512), ping-pong a0/a1 ----
            for layer in range(D):
                src = abuf[layer % 2]
                dst = abuf[(layer + 1) % 2]
                for co in range(NCO):
                    wt = wmid_p.tile([128, K, NCI, 128], F32R, tag="wm")
                    nc.sync.dma_start(out=wt[:], in_=w_mid[layer, co])
                    for t in range(NT):
                        ps = ps_p.tile([128, TT], F32, tag="ps")
                        n = 0
                        for ci in range(NCI):
                            for k in range(K):
                                nc.tensor.matmul(
                                    ps[:],
                                    wt[:, k, ci, :],
                                    src[:, ci, t * TT + k: t * TT + k + TT],
                                    start=(n == 0), stop=(n == NCI * K - 1))
                                n += 1
                        nc.scalar.activation(
                            out=dst[:, co, PAD + t * TT: PAD + (t + 1) * TT],
                            in_=ps[:], func=Relu,
                            bias=bim[:, layer, co, :], scale=scm[:, layer, co, :])

            # ---- layer 5: out conv (512 -> 257) from a0 (D even) ----
            src = abuf[D % 2]
            for co in range(2):
                wt = wmid_p.tile([128, K, NCI, 128], F32R, tag="wm")
                nc.sync.dma_start(out=wt[:], in_=w_out_a[co])
                for t in range(NT):
                    ps = ps_p.tile([128, TT], F32, tag="ps")
                    n = 0
                    for ci in range(NCI):
                        for k in range(K):
                            nc.tensor.matmul(
                                ps[:],
                                wt[:, k, ci, :],
                                src[:, ci, t * TT + k: t * TT + k + TT],
                                start=(n == 0), stop=(n == NCI * K - 1))
                            n += 1
                    ot = out_p.tile([128, TT], F32, tag="ot")
                    nc.scalar.activation(
                        out=ot[:], in_=ps[:], func=Relu,
                        bias=bio[:, co, :], scale=sco[:, co, :])
                    nc.sync.dma_start(
                        out=y[co * 128:(co + 1) * 128, t * TT:(t + 1) * TT],
                        in_=ot[:])
            # output channel 256 (M=1 matmuls)
            wtb = win_p.tile([128, K, NCI, 1], F32R, tag="wtb")
            nc.sync.dma_start(out=wtb[:], in_=w_out_b[:])
            for t in range(NT):
                psb = psb_p.tile([1, TT], F32, tag="psb")
                n = 0
                for ci in range(NCI):
                    for k in range(K):
                        nc.tensor.matmul(
                            psb[:],
                            wtb[:, k, ci, :],
                            src[:, ci, t * TT + k: t * TT + k + TT],
                            start=(n == 0), stop=(n == NCI * K - 1))
                        n += 1
                otb = outb_p.tile([1, TT], F32, tag="otb")
                nc.scalar.activation(
                    out=otb[:], in_=psb[:], func=Relu,
                    bias=biob[:], scale=scob[:])
                nc.sync.dma_start(
                    out=y[256:257, t * TT:(t + 1) * TT], in_=otb[:])

    nc.compile()
    _NC_CACHE['nc'] = nc
    return nc


def prep_inputs(inputs):
    """Host-side prep: clip weights, fold BN, transpose, shard per core.

    Returns list of 8 per-core input dicts."""
    # force plain numpy up front (harness may hand us jax arrays)
    inputs = {k: np.asarray(v) for k, v in inputs.items()}
    noisy = np.ascontiguousarray(inputs["noisy"], dtype=np.float32)

    w_in = np.clip(inputs["w_in"].astype(np.float32), -1.0, 1.0)
    w_mid = np.clip(inputs["w_mid"].astype(np.float32), -1.0, 1.0)
    w_out = np.clip(inputs["w_out"].astype(np.float32), -1.0, 1.0)

    # transpose to [k, ci, co], then pre-tile to partition-contiguous chunk
    # layouts: [co_blk, p(=ci%128), k, ci_blk, co_in]
    w_in_t = w_in.transpose(2, 1, 0)                             # [9, 257, 512]
    w_in_a = np.ascontiguousarray(
        w_in_t[:, :256, :].reshape(K, 2, 128, NCO, 128).transpose(3, 2, 0, 1, 4))
    w_in_b = np.zeros((128, C), dtype=np.float32)                # padded [128, 512]
    w_in_b[:K] = w_in_t[:, 256, :]
    w_mid_t = np.ascontiguousarray(
        w_mid.transpose(0, 3, 2, 1)                              # [4, 9, 512, 512]
        .reshape(D, K, NCI, 128, NCO, 128).transpose(0, 4, 3, 1, 2, 5))
    w_out_t = w_out.transpose(2, 1, 0)                           # [9, 512, 257]
    w_out_a = np.ascontiguousarray(
        w_out_t[:, :, :256].reshape(K, NCI, 128, 2, 128).transpose(3, 2, 0, 1, 4))
    w_out_b = np.ascontiguousarray(
        w_out_t[:, :, 256].reshape(K, NCI, 128).transpose(2, 0, 1))[..., None]

    def fold(gamma, beta, mean, var):
        g = gamma.astype(np.float32)
        inv = g / np.sqrt(var.astype(np.float32) + np.float32(BN_EPS))
        bias = beta.astype(np.float32) - mean.astype(np.float32) * inv
        return inv.astype(np.float32), bias.astype(np.float32)

    sc_in_v, bi_in_v = fold(inputs["bn_in_gamma"], inputs["bn_in_beta"],
                            inputs["bn_in_mean"], inputs["bn_in_var"])
    sc_mid_v, bi_mid_v = fold(inputs["bn_mid_gamma"], inputs["bn_mid_beta"],
                              inputs["bn_mid_mean"], inputs["bn_mid_var"])
    sc_out_v, bi_out_v = fold(inputs["bn_out_gamma"], inputs["bn_out_beta"],
                              inputs["bn_out_mean"], inputs["bn_out_var"])

    tab = np.zeros((128, 46), dtype=np.float32)
    tab[:, 0:4] = sc_in_v.reshape(4, 128).T          # [p, b]
    tab[:, 4:8] = bi_in_v.reshape(4, 128).T
    tab[:, 8:24] = sc_mid_v.reshape(D * 4, 128).T    # [p, l*b]
    tab[:, 24:40] = bi_mid_v.reshape(D * 4, 128).T
    tab[:, 40:42] = sc_out_v[:256].reshape(2, 128).T
    tab[:, 42:44] = bi_out_v[:256].reshape(2, 128).T
    tab[0, 44] = sc_out_v[256]
    tab[0, 45] = bi_out_v[256]

    shared = {
        "w_in_a": w_in_a, "w_in_b": w_in_b, "w_mid": w_mid_t,
        "w_out_a": w_out_a, "w_out_b": w_out_b, "sbtab": tab,
    }

    in_maps = []
    for b in range(B):
        x_main = np.ascontiguousarray(noisy[b, :256, :])         # [256, 2000]
        xp = np.zeros(TP, dtype=np.float32)
        xp[PAD:PAD + T] = noisy[b, 256, :]
        xk9 = np.zeros((K, TP), dtype=np.float32)
        for k in range(K):
            xk9[k, :TP - k] = xp[k:]
        in_maps.append(dict(shared, x_main=x_main, xk9=xk9))
    return in_maps


def run(inputs, **run_kwargs):
    nc = build_nc()
    in_maps = prep_inputs(inputs)
    res = run_bass_kernel_spmd(nc, in_maps, core_ids=list(range(B)), **run_kwargs)
    out = np.stack([r["y"] for r in res.results]).astype(np.float32)
    return out, res


def kernel(**inputs) -> np.ndarray:
    out, _ = run(inputs)
    return out


# revision 12
# speedup vs baseline: 1.0010x; 1.0010x over previous
"""Trainium2 Bass kernel for nn_Network_75084618269290 (dense_cnn).

Network: 6x conv1d(SAME, K=9) + BN + ReLU stack:
  [B=8, 257, 2000] -> conv(257->512) -> 4x conv(512->512) -> conv(512->257)
Weights are clamped to [-1, 1] (done host-side), BN is inference-mode
(folded host-side into per-channel scale/bias).

Sharding: data-parallel over batch B=8 across the 8 NeuronCores -- one
sample per core, zero collectives. Each conv is computed as 1d-conv =
sum over (ci_block, tap) of 128-contraction matmuls accumulated in PSUM,
with a fused scale+bias+ReLU PSUM->SBUF eviction on the scalar engine.
Matmuls run in float32r (full PE rate at N>=256, ~1e-4 rel err).

Self-contained: hardcodes all shapes; host-side prep = clip + transpose +
BN fold + per-sample sharding.
"""
import sys

for _p in ('/opt/trn_rl_repo',):
    if _p not in sys.path:
        sys.path.insert(0, _p)

import numpy as np

import concourse.bass as bass  # noqa: F401  (bass types used via bacc)
from concourse import bacc
import concourse.tile as tile
import concourse.mybir as mybir
from concourse.bass_utils import run_bass_kernel_spmd

# ---- problem constants (hardcoded per contract) ----
B = 8
INP = 257
C = 512
D = 4
K = 9
T = 2000
PAD = 4
TT = 500              # t-tile (PSUM bank holds 512 fp32)
NT = T // TT          # 4
TP = T + 2 * PAD      # 2008 padded time axis
NCI = C // 128        # 4
NCO = C // 128        # 4
BN_EPS = 1e-5

F32R = mybir.dt.float32r
F32 = mybir.dt.float32

_NC_CACHE = {}


def build_nc():
    """Build + compile the per-core Bass program (SPMD: same NEFF on all 8)."""
    if 'nc' in _NC_CACHE:
        return _NC_CACHE['nc']

    nc = bacc.Bacc(None, target_bir_lowering=False)

    # ---- DRAM I/O ----
    x_main = nc.dram_tensor("x_main", [256, T], F32R, kind="ExternalInput")
    xk9 = nc.dram_tensor("xk9", [K, TP], F32R, kind="ExternalInput")
    # weight layouts are pre-tiled host-side so every per-chunk DMA is
    # partition-contiguous (one long line per partition -> few descriptors)
    w_in_a = nc.dram_tensor("w_in_a", [NCO, 128, K, 2, 128], F32R, kind="ExternalInput")
    w_in_b = nc.dram_tensor("w_in_b", [128, C], F32R, kind="ExternalInput")
    w_mid = nc.dram_tensor("w_mid", [D, NCO, 128, K, NCI, 128], F32R, kind="ExternalInput")
    w_out_a = nc.dram_tensor("w_out_a", [2, 128, K, NCI, 128], F32R, kind="ExternalInput")
    w_out_b = nc.dram_tensor("w_out_b", [128, K, NCI, 1], F32R, kind="ExternalInput")
    # packed BN table: per-partition columns =
    # [sc_in(4), bi_in(4), sc_mid(16), bi_mid(16), sc_out(2), bi_out(2),
    #  sc_out_b(1: row0 only), bi_out_b(1: row0 only)]
    sbtab = nc.dram_tensor("sbtab", [128, 46], F32, kind="ExternalInput")
    y = nc.dram_tensor("y", [INP, T], F32, kind="ExternalOutput")

    Relu = mybir.ActivationFunctionType.Relu

    with tile.TileContext(nc) as tc:
        with tc.tile_pool(name="xin", bufs=1) as xin_p, \
             tc.tile_pool(name="act", bufs=1) as act_p, \
             tc.tile_pool(name="win", bufs=1) as win_p, \
             tc.tile_pool(name="wmid", bufs=2) as wmid_p, \
             tc.tile_pool(name="sb", bufs=1) as sb_p, \
             tc.tile_pool(name="ps", bufs=6, space="PSUM") as ps_p, \
             tc.tile_pool(name="psb", bufs=2, space="PSUM") as psb_p, \
             tc.tile_pool(name="out", bufs=4) as out_p, \
             tc.tile_pool(name="outb", bufs=2) as outb_p:

            # PE pre-warm: dummy matmuls on zeroed scratch keep the PE busy
            # during the initial DMA so HAM un-throttles (1.2->2.4GHz) before
            # the first real matmul group. Scratch memset goes to the idle
            # GpSimd engine so the warm matmuls start as early as possible.
            warm = xin_p.tile([128, 512], F32R, tag="warm")
            nc.gpsimd.memset(warm.bitcast(F32), 0.0)
            warm_ps = ps_p.tile([128, TT], F32, tag="ps")
            for _ in range(12):
                nc.tensor.matmul(warm_ps[:], warm[:, 0:128], warm[:, 0:TT],
                                 start=True, stop=True)

            # ---- input buffer: memset only the pad columns (critical path) ----
            xin = xin_p.tile([128, 2, TP], F32R, tag="xin")
            nc.vector.memset(xin[:, :, 0:PAD].bitcast(F32), 0.0)
            nc.vector.memset(xin[:, :, PAD + T:TP].bitcast(F32), 0.0)
            # split the x DMA so the first matmul group (t=0, cols < 512) can
            # start after the first chunk lands
            x_r = x_main.rearrange("(b p) t -> p b t", p=128)
            XSPL = 504
            # sync HW queue drains FIFO: put exactly what the first 9 matmuls
            # need up front (x ci-block 0 cols<512, wi chunk 0 ci-half 0),
            # then the rest in need order.
            nc.sync.dma_start(out=xin[:, 0:1, PAD:PAD + XSPL], in_=x_r[:, 0:1, 0:XSPL])
            wi0 = win_p.tile([128, K, 2, 128], F32R, tag="wi0")
            nc.sync.dma_start(out=wi0[:, :, 0:1, :], in_=w_in_a[0, :, :, 0:1, :])
            nc.sync.dma_start(out=xin[:, 1:2, PAD:PAD + XSPL], in_=x_r[:, 1:2, 0:XSPL])
            nc.sync.dma_start(out=wi0[:, :, 1:2, :], in_=w_in_a[0, :, :, 1:2, :])
            # xk9/wib padded to full 128-row contraction (zero rows 9..127) so
            # the channel-256 matmul uses all row groups and pipelines cleanly
            xk9t = xin_p.tile([128, TP], F32R, tag="xk9")
            nc.vector.memset(xk9t.bitcast(F32), 0.0)
            nc.sync.dma_start(out=xk9t[0:K, :], in_=xk9[:])
            wib = win_p.tile([128, C], F32R, tag="wib")
            nc.sync.dma_start(out=wib[:], in_=w_in_b[:])

            wi_t = [wi0]
            for co in range(1, NCO):
                wic = win_p.tile([128, K, 2, 128], F32R, tag=f"wi{co}")
                nc.sync.dma_start(out=wic[:], in_=w_in_a[co])
                wi_t.append(wic)
            # rest of x lands after the in-layer weight chunks: with the
            # t-outer in-layer loop below, nothing needs it until ~30us
            nc.sync.dma_start(out=xin[:, :, PAD + XSPL:PAD + T], in_=x_r[:, :, XSPL:T])

            a0 = act_p.tile([128, NCI, TP], F32R, tag="a0")
            a1 = act_p.tile([128, NCI, TP], F32R, tag="a1")
            nc.vector.memset(a0[:, :, 0:PAD].bitcast(F32), 0.0)
            nc.vector.memset(a0[:, :, PAD + T:TP].bitcast(F32), 0.0)
            nc.vector.memset(a1[:, :, 0:PAD].bitcast(F32), 0.0)
            nc.vector.memset(a1[:, :, PAD + T:TP].bitcast(F32), 0.0)
            abuf = [a0, a1]

            tab = sb_p.tile([128, 46], F32, tag="tab")
            nc.scalar.dma_start(out=tab[:], in_=sbtab[:])
            sci = tab[:, 0:4].rearrange("p (b o) -> p b o", o=1)
            bii = tab[:, 4:8].rearrange("p (b o) -> p b o", o=1)
            scm = tab[:, 8:24].rearrange("p (l b o) -> p l b o", l=D, o=1)
            bim = tab[:, 24:40].rearrange("p (l b o) -> p l b o", l=D, o=1)
            sco = tab[:, 40:42].rearrange("p (b o) -> p b o", o=1)
            bio = tab[:, 42:44].rearrange("p (b o) -> p b o", o=1)
            scob = tab[0:1, 44:45]
            biob = tab[0:1, 45:46]

            # ---- layer 0: in conv (257 -> 512), writes a0 ----
            # t-outer: the four t=0 groups only need the first x chunk, so the
            # PE never waits for the bulk x DMA
            for t in range(NT):
                for co in range(NCO):
                    ps = ps_p.tile([128, TT], F32, tag="ps")
                    n = 0
                    nmm = 2 * K + 1
                    for ci in range(2):
                        for k in range(K):
                            nc.tensor.matmul(
                                ps[:],
                                wi_t[co][:, k, ci, :],
                                xin[:, ci, t * TT + k: t * TT + k + TT],
                                start=(n == 0), stop=(n == nmm - 1))
                            n += 1
                    # channel 256 via K=9 contraction on host-im2col'd row
                    nc.tensor.matmul(
                        ps[:],
                        wib[:, co * 128:(co + 1) * 128],
                        xk9t[:, t * TT: t * TT + TT],
                        start=False, stop=True)
                    nc.scalar.activation(
                        out=a0[:, co, PAD + t * TT: PAD + (t + 1) * TT],
                        in_=ps[:], func=Relu,
                        bias=bii[:, co, :], scale=sci[:, co, :])

            # ---- layers 1..4: mid convs (512 -> 512), ping-pong a0/a1 ----
            for layer in range(D):
                src = abuf[layer % 2]
                dst = abuf[(layer + 1) % 2]
                for co in range(NCO):
                    wt = wmid_p.tile([128, K, NCI, 128], F32R, tag="wm")
                    nc.sync.dma_start(out=wt[:], in_=w_mid[layer, co])
                    for t in range(NT):
                        ps = ps_p.tile([128, TT], F32, tag="ps")
                        n = 0
                        for ci in range(NCI):
                            for k in range(K):
                                nc.tensor.matmul(
                                    ps[:],
                                    wt[:, k, ci, :],
                                    src[:, ci, t * TT + k: t * TT + k + TT],
                                    start=(n == 0), stop=(n == NCI * K - 1))
                                n += 1
                        nc.scalar.activation(
                            out=dst[:, co, PAD + t * TT: PAD + (t + 1) * TT],
                            in_=ps[:], func=Relu,
                            bias=bim[:, layer, co, :], scale=scm[:, layer, co, :])

            # ---- layer 5: out conv (512 -> 257) from a0 (D even) ----
            src = abuf[D % 2]
            for co in range(2):
                wt = wmid_p.tile([128, K, NCI, 128], F32R, tag="wm")
                nc.sync.dma_start(out=wt[:], in_=w_out_a[co])
                for t in range(NT):
                    ps = ps_p.tile([128, TT], F32, tag="ps")
                    n = 0
                    for ci in range(NCI):
                        for k in range(K):
                            nc.tensor.matmul(
                                ps[:],
                                wt[:, k, ci, :],
                                src[:, ci, t * TT + k: t * TT + k + TT],
                                start=(n == 0), stop=(n == NCI * K - 1))
                            n += 1
                    ot = out_p.tile([128, TT], F32, tag="ot")
                    nc.scalar.activation(
                        out=ot[:], in_=ps[:], func=Relu,
                        bias=bio[:, co, :], scale=sco[:, co, :])
                    nc.sync.dma_start(
                        out=y[co * 128:(co + 1) * 128, t * TT:(t + 1) * TT],
                        in_=ot[:])
            # output channel 256 (M=1 matmuls)
            wtb = win_p.tile([128, K, NCI, 1], F32R, tag="wtb")
            nc.sync.dma_start(out=wtb[:], in_=w_out_b[:])
            for t in range(NT):
                psb = psb_p.tile([1, TT], F32, tag="psb")
                n = 0
                for ci in range(NCI):
                    for k in range(K):
                        nc.tensor.matmul(
                            psb[:],
                            wtb[:, k, ci, :],
                            src[:, ci, t * TT + k: t * TT + k + TT],
                            start=(n == 0), stop=(n == NCI * K - 1))
                        n += 1
                otb = outb_p.tile([1, TT], F32, tag="otb")
                nc.scalar.activation(
                    out=otb[:], in_=psb[:], func=Relu,
                    bias=biob[:], scale=scob[:])
                nc.sync.dma_start(
                    out=y[256:257, t * TT:(t + 1) * TT], in_=otb[:])

    nc.compile()
    _NC_CACHE['nc'] = nc
    return nc


def prep_inputs(inputs):
    """Host-side prep: clip weights, fold BN, transpose, shard per core.

    Returns list of 8 per-core input dicts."""
    # force plain numpy up front (harness may hand us jax arrays)
    inputs = {k: np.asarray(v) for k, v in inputs.items()}
    noisy = np.ascontiguousarray(inputs["noisy"], dtype=np.float32)

    w_in = np.clip(inputs["w_in"].astype(np.float32), -1.0, 1.0)
    w_mid = np.clip(inputs["w_mid"].astype(np.float32), -1.0, 1.0)
    w_out = np.clip(inputs["w_out"].astype(np.float32), -1.0, 1.0)

    # transpose to [k, ci, co], then pre-tile to partition-contiguous chunk
    # layouts: [co_blk, p(=ci%128), k, ci_blk, co_in]
    w_in_t = w_in.transpose(2, 1, 0)                             # [9, 257, 512]
    w_in_a = np.ascontiguousarray(
        w_in_t[:, :256, :].reshape(K, 2, 128, NCO, 128).transpose(3, 2, 0, 1, 4))
    w_in_b = np.zeros((128, C), dtype=np.float32)                # padded [128, 512]
    w_in_b[:K] = w_in_t[:, 256, :]
    w_mid_t = np.ascontiguousarray(
        w_mid.transpose(0, 3, 2, 1)                              # [4, 9, 512, 512]
        .reshape(D, K, NCI, 128, NCO, 128).transpose(0, 4, 3, 1, 2, 5))
    w_out_t = w_out.transpose(2, 1, 0)                           # [9, 512, 257]
    w_out_a = np.ascontiguousarray(
        w_out_t[:, :, :256].reshape(K, NCI, 128, 2, 128).transpose(3, 2, 0, 1, 4))
    w_out_b = np.ascontiguousarray(
        w_out_t[:, :, 256].reshape(K, NCI, 128).transpose(2, 0, 1))[..., None]

    def fold(gamma, beta, mean, var):
        g = gamma.astype(np.float32)
        inv = g / np.sqrt(var.astype(np.float32) + np.float32(BN_EPS))
        bias = beta.astype(np.float32) - mean.astype(np.float32) * inv
        return inv.astype(np.float32), bias.astype(np.float32)

    sc_in_v, bi_in_v = fold(inputs["bn_in_gamma"], inputs["bn_in_beta"],
                            inputs["bn_in_mean"], inputs["bn_in_var"])
    sc_mid_v, bi_mid_v = fold(inputs["bn_mid_gamma"], inputs["bn_mid_beta"],
                              inputs["bn_mid_mean"], inputs["bn_mid_var"])
    sc_out_v, bi_out_v = fold(inputs["bn_out_gamma"], inputs["bn_out_beta"],
                              inputs["bn_out_mean"], inputs["bn_out_var"])

    tab = np.zeros((128, 46), dtype=np.float32)
    tab[:, 0:4] = sc_in_v.reshape(4, 128).T          # [p, b]
    tab[:, 4:8] = bi_in_v.reshape(4, 128).T
    tab[:, 8:24] = sc_mid_v.reshape(D * 4, 128).T    # [p, l*b]
    tab[:, 24:40] = bi_mid_v.reshape(D * 4, 128).T
    tab[:, 40:42] = sc_out_v[:256].reshape(2, 128).T
    tab[:, 42:44] = bi_out_v[:256].reshape(2, 128).T
    tab[0, 44] = sc_out_v[256]
    tab[0, 45] = bi_out_v[256]

    shared = {
        "w_in_a": w_in_a, "w_in_b": w_in_b, "w_mid": w_mid_t,
        "w_out_a": w_out_a, "w_out_b": w_out_b, "sbtab": tab,
    }

    in_maps = []
    for b in range(B):
        x_main = np.ascontiguousarray(noisy[b, :256, :])         # [256, 2000]
        xp = np.zeros(TP, dtype=np.float32)
        xp[PAD:PAD + T] = noisy[b, 256, :]
        xk9 = np.zeros((K, TP), dtype=np.float32)
        for k in range(K):
            xk9[k, :TP - k] = xp[k:]
        in_maps.append(dict(shared, x_main=x_main, xk9=xk9))
    return in_maps


def run(inputs, **run_kwargs):
    nc = build_nc()
    in_maps = prep_inputs(inputs)
    res = run_bass_kernel_spmd(nc, in_maps, core_ids=list(range(B)), **run_kwargs)
    out = np.stack([r["y"] for r in res.results]).astype(np.float32)
    return out, res


def kernel(**inputs) -> np.ndarray:
    out, _ = run(inputs)
    return out


# revision 13
# speedup vs baseline: 1.0029x; 1.0019x over previous
"""Trainium2 Bass kernel for nn_Network_75084618269290 (dense_cnn).

Network: 6x conv1d(SAME, K=9) + BN + ReLU stack:
  [B=8, 257, 2000] -> conv(257->512) -> 4x conv(512->512) -> conv(512->257)
Weights are clamped to [-1, 1] (done host-side), BN is inference-mode
(folded host-side into per-channel scale/bias).

Sharding: data-parallel over batch B=8 across the 8 NeuronCores -- one
sample per core, zero collectives. Each conv is computed as 1d-conv =
sum over (ci_block, tap) of 128-contraction matmuls accumulated in PSUM,
with a fused scale+bias+ReLU PSUM->SBUF eviction on the scalar engine.
Matmuls run in float32r (full PE rate at N>=256, ~1e-4 rel err).

Self-contained: hardcodes all shapes; host-side prep = clip + transpose +
BN fold + per-sample sharding.
"""
import sys

for _p in ('/opt/trn_rl_repo',):
    if _p not in sys.path:
        sys.path.insert(0, _p)

import numpy as np

import concourse.bass as bass  # noqa: F401  (bass types used via bacc)
from concourse import bacc
import concourse.tile as tile
import concourse.mybir as mybir
from concourse.bass_utils import run_bass_kernel_spmd

# ---- problem constants (hardcoded per contract) ----
B = 8
INP = 257
C = 512
D = 4
K = 9
T = 2000
PAD = 4
TT = 500              # t-tile (PSUM bank holds 512 fp32)
NT = T // TT          # 4
TP = T + 2 * PAD      # 2008 padded time axis
NCI = C // 128        # 4
NCO = C // 128        # 4
BN_EPS = 1e-5

F32R = mybir.dt.float32r
F32 = mybir.dt.float32

_NC_CACHE = {}


def build_nc():
    """Build + compile the per-core Bass program (SPMD: same NEFF on all 8)."""
    if 'nc' in _NC_CACHE:
        return _NC_CACHE['nc']

    nc = bacc.Bacc(None, target_bir_lowering=False)

    # ---- DRAM I/O ----
    x_main = nc.dram_tensor("x_main", [256, T], F32R, kind="ExternalInput")
    xk9 = nc.dram_tensor("xk9", [K, TP], F32R, kind="ExternalInput")
    # weight layouts are pre-tiled host-side so every per-chunk DMA is
    # partition-contiguous (one long line per partition -> few descriptors)
    w_in_a = nc.dram_tensor("w_in_a", [NCO, 128, K, 2, 128], F32R, kind="ExternalInput")
    w_in_b = nc.dram_tensor("w_in_b", [128, C], F32R, kind="ExternalInput")
    w_mid = nc.dram_tensor("w_mid", [D, NCO, 128, K, NCI, 128], F32R, kind="ExternalInput")
    w_out_a = nc.dram_tensor("w_out_a", [2, 128, K, NCI, 128], F32R, kind="ExternalInput")
    w_out_b = nc.dram_tensor("w_out_b", [128, K, NCI, 1], F32R, kind="ExternalInput")
    # packed BN table: per-partition columns =
    # [sc_in(4), bi_in(4), sc_mid(16), bi_mid(16), sc_out(2), bi_out(2),
    #  sc_out_b(1: row0 only), bi_out_b(1: row0 only)]
    sbtab = nc.dram_tensor("sbtab", [128, 46], F32, kind="ExternalInput")
    y = nc.dram_tensor("y", [INP, T], F32, kind="ExternalOutput")

    Relu = mybir.ActivationFunctionType.Relu

    with tile.TileContext(nc) as tc:
        with tc.tile_pool(name="xin", bufs=1) as xin_p, \
             tc.tile_pool(name="act", bufs=1) as act_p, \
             tc.tile_pool(name="win", bufs=1) as win_p, \
             tc.tile_pool(name="wmid", bufs=2) as wmid_p, \
             tc.tile_pool(name="sb", bufs=1) as sb_p, \
             tc.tile_pool(name="ps", bufs=6, space="PSUM") as ps_p, \
             tc.tile_pool(name="psb", bufs=2, space="PSUM") as psb_p, \
             tc.tile_pool(name="out", bufs=4) as out_p, \
             tc.tile_pool(name="outb", bufs=2) as outb_p:

            # PE pre-warm: dummy matmuls on zeroed scratch keep the PE busy
            # during the initial DMA so HAM un-throttles (1.2->2.4GHz) before
            # the first real matmul group. Scratch memset goes to the idle
            # GpSimd engine so the warm matmuls start as early as possible.
            warm = xin_p.tile([128, 512], F32R, tag="warm")
            nc.gpsimd.memset(warm.bitcast(F32), 1.0)
            warm_ps = ps_p.tile([128, TT], F32, tag="ps")
            for _ in range(12):
                nc.tensor.matmul(warm_ps[:], warm[:, 0:128], warm[:, 0:TT],
                                 start=True, stop=True)

            # ---- input buffer: memset only the pad columns (critical path) ----
            xin = xin_p.tile([128, 2, TP], F32R, tag="xin")
            nc.vector.memset(xin[:, :, 0:PAD].bitcast(F32), 0.0)
            nc.vector.memset(xin[:, :, PAD + T:TP].bitcast(F32), 0.0)
            # split the x DMA so the first matmul group (t=0, cols < 512) can
            # start after the first chunk lands
            x_r = x_main.rearrange("(b p) t -> p b t", p=128)
            XSPL = 504
            # sync HW queue drains FIFO: put exactly what the first 9 matmuls
            # need up front (x ci-block 0 cols<512, wi chunk 0 ci-half 0),
            # then the rest in need order.
            nc.sync.dma_start(out=xin[:, 0:1, PAD:PAD + XSPL], in_=x_r[:, 0:1, 0:XSPL])
            wi0 = win_p.tile([128, K, 2, 128], F32R, tag="wi0")
            nc.sync.dma_start(out=wi0[:, :, 0:1, :], in_=w_in_a[0, :, :, 0:1, :])
            nc.sync.dma_start(out=xin[:, 1:2, PAD:PAD + XSPL], in_=x_r[:, 1:2, 0:XSPL])
            nc.sync.dma_start(out=wi0[:, :, 1:2, :], in_=w_in_a[0, :, :, 1:2, :])
            # xk9/wib padded to full 128-row contraction (zero rows 9..127) so
            # the channel-256 matmul uses all row groups and pipelines cleanly
            xk9t = xin_p.tile([128, TP], F32R, tag="xk9")
            nc.vector.memset(xk9t.bitcast(F32), 0.0)
            nc.sync.dma_start(out=xk9t[0:K, :], in_=xk9[:])
            # wi1 before wib: co=1's group starts at nearly the same time the
            # co=0 group finishes, so its weights must not queue behind wib
            wi1 = win_p.tile([128, K, 2, 128], F32R, tag="wi1")
            nc.sync.dma_start(out=wi1[:], in_=w_in_a[1])
            wib = win_p.tile([128, C], F32R, tag="wib")
            nc.sync.dma_start(out=wib[:], in_=w_in_b[:])

            wi_t = [wi0, wi1]
            for co in range(2, NCO):
                wic = win_p.tile([128, K, 2, 128], F32R, tag=f"wi{co}")
                nc.sync.dma_start(out=wic[:], in_=w_in_a[co])
                wi_t.append(wic)
            # rest of x lands after the in-layer weight chunks: with the
            # t-outer in-layer loop below, nothing needs it until ~30us
            nc.sync.dma_start(out=xin[:, :, PAD + XSPL:PAD + T], in_=x_r[:, :, XSPL:T])

            a0 = act_p.tile([128, NCI, TP], F32R, tag="a0")
            a1 = act_p.tile([128, NCI, TP], F32R, tag="a1")
            nc.vector.memset(a0[:, :, 0:PAD].bitcast(F32), 0.0)
            nc.vector.memset(a0[:, :, PAD + T:TP].bitcast(F32), 0.0)
            nc.vector.memset(a1[:, :, 0:PAD].bitcast(F32), 0.0)
            nc.vector.memset(a1[:, :, PAD + T:TP].bitcast(F32), 0.0)
            abuf = [a0, a1]

            tab = sb_p.tile([128, 46], F32, tag="tab")
            nc.scalar.dma_start(out=tab[:], in_=sbtab[:])
            sci = tab[:, 0:4].rearrange("p (b o) -> p b o", o=1)
            bii = tab[:, 4:8].rearrange("p (b o) -> p b o", o=1)
            scm = tab[:, 8:24].rearrange("p (l b o) -> p l b o", l=D, o=1)
            bim = tab[:, 24:40].rearrange("p (l b o) -> p l b o", l=D, o=1)
            sco = tab[:, 40:42].rearrange("p (b o) -> p b o", o=1)
            bio = tab[:, 42:44].rearrange("p (b o) -> p b o", o=1)
            scob = tab[0:1, 44:45]
            biob = tab[0:1, 45:46]

            # ---- layer 0: in conv (257 -> 512), writes a0 ----
            # t-outer: the four t=0 groups only need the first x chunk, so the
            # PE never waits for the bulk x DMA
            for t in range(NT):
                for co in range(NCO):
                    ps = ps_p.tile([128, TT], F32, tag="ps")
                    n = 0
                    nmm = 2 * K + 1
                    for ci in range(2):
                        for k in range(K):
                            nc.tensor.matmul(
                                ps[:],
                                wi_t[co][:, k, ci, :],
                                xin[:, ci, t * TT + k: t * TT + k + TT],
                                start=(n == 0), stop=(n == nmm - 1))
                            n += 1
                    # channel 256 via K=9 contraction on host-im2col'd row
                    nc.tensor.matmul(
                        ps[:],
                        wib[:, co * 128:(co + 1) * 128],
                        xk9t[:, t * TT: t * TT + TT],
                        start=False, stop=True)
                    nc.scalar.activation(
                        out=a0[:, co, PAD + t * TT: PAD + (t + 1) * TT],
                        in_=ps[:], func=Relu,
                        bias=bii[:, co, :], scale=sci[:, co, :])

            # ---- layers 1..4: mid convs (512 -> 512), ping-pong a0/a1 ----
            for layer in range(D):
                src = abuf[layer % 2]
                dst = abuf[(layer + 1) % 2]
                for co in range(NCO):
                    wt = wmid_p.tile([128, K, NCI, 128], F32R, tag="wm")
                    nc.sync.dma_start(out=wt[:], in_=w_mid[layer, co])
                    for t in range(NT):
                        ps = ps_p.tile([128, TT], F32, tag="ps")
                        n = 0
                        for ci in range(NCI):
                            for k in range(K):
                                nc.tensor.matmul(
                                    ps[:],
                                    wt[:, k, ci, :],
                                    src[:, ci, t * TT + k: t * TT + k + TT],
                                    start=(n == 0), stop=(n == NCI * K - 1))
                                n += 1
                        nc.scalar.activation(
                            out=dst[:, co, PAD + t * TT: PAD + (t + 1) * TT],
                            in_=ps[:], func=Relu,
                            bias=bim[:, layer, co, :], scale=scm[:, layer, co, :])

            # ---- layer 5: out conv (512 -> 257) from a0 (D even) ----
            src = abuf[D % 2]
            for co in range(2):
                wt = wmid_p.tile([128, K, NCI, 128], F32R, tag="wm")
                nc.sync.dma_start(out=wt[:], in_=w_out_a[co])
                for t in range(NT):
                    ps = ps_p.tile([128, TT], F32, tag="ps")
                    n = 0
                    for ci in range(NCI):
                        for k in range(K):
                            nc.tensor.matmul(
                                ps[:],
                                wt[:, k, ci, :],
                                src[:, ci, t * TT + k: t * TT + k + TT],
                                start=(n == 0), stop=(n == NCI * K - 1))
                            n += 1
                    ot = out_p.tile([128, TT], F32, tag="ot")
                    nc.scalar.activation(
                        out=ot[:], in_=ps[:], func=Relu,
                        bias=bio[:, co, :], scale=sco[:, co, :])
                    nc.sync.dma_start(
                        out=y[co * 128:(co + 1) * 128, t * TT:(t + 1) * TT],
                        in_=ot[:])
            # output channel 256 (M=1 matmuls)
            wtb = win_p.tile([128, K, NCI, 1], F32R, tag="wtb")
            nc.sync.dma_start(out=wtb[:], in_=w_out_b[:])
            for t in range(NT):
                psb = psb_p.tile([1, TT], F32, tag="psb")
                n = 0
                for ci in range(NCI):
                    for k in range(K):
                        nc.tensor.matmul(
                            psb[:],
                            wtb[:, k, ci, :],
                            src[:, ci, t * TT + k: t * TT + k + TT],
                            start=(n == 0), stop=(n == NCI * K - 1))
                        n += 1
                otb = outb_p.tile([1, TT], F32, tag="otb")
                nc.scalar.activation(
                    out=otb[:], in_=psb[:], func=Relu,
                    bias=biob[:], scale=scob[:])
                nc.sync.dma_start(
                    out=y[256:257, t * TT:(t + 1) * TT], in_=otb[:])

    nc.compile()
    _NC_CACHE['nc'] = nc
    return nc


def prep_inputs(inputs):
    """Host-side prep: clip weights, fold BN, transpose, shard per core.

    Returns list of 8 per-core input dicts."""
    # force plain numpy up front (harness may hand us jax arrays)
    inputs = {k: np.asarray(v) for k, v in inputs.items()}
    noisy = np.ascontiguousarray(inputs["noisy"], dtype=np.float32)

    w_in = np.clip(inputs["w_in"].astype(np.float32), -1.0, 1.0)
    w_mid = np.clip(inputs["w_mid"].astype(np.float32), -1.0, 1.0)
    w_out = np.clip(inputs["w_out"].astype(np.float32), -1.0, 1.0)

    # transpose to [k, ci, co], then pre-tile to partition-contiguous chunk
    # layouts: [co_blk, p(=ci%128), k, ci_blk, co_in]
    w_in_t = w_in.transpose(2, 1, 0)                             # [9, 257, 512]
    w_in_a = np.ascontiguousarray(
        w_in_t[:, :256, :].reshape(K, 2, 128, NCO, 128).transpose(3, 2, 0, 1, 4))
    w_in_b = np.zeros((128, C), dtype=np.float32)                # padded [128, 512]
    w_in_b[:K] = w_in_t[:, 256, :]
    w_mid_t = np.ascontiguousarray(
        w_mid.transpose(0, 3, 2, 1)                              # [4, 9, 512, 512]
        .reshape(D, K, NCI, 128, NCO, 128).transpose(0, 4, 3, 1, 2, 5))
    w_out_t = w_out.transpose(2, 1, 0)                           # [9, 512, 257]
    w_out_a = np.ascontiguousarray(
        w_out_t[:, :, :256].reshape(K, NCI, 128, 2, 128).transpose(3, 2, 0, 1, 4))
    w_out_b = np.ascontiguousarray(
        w_out_t[:, :, 256].reshape(K, NCI, 128).transpose(2, 0, 1))[..., None]

    def fold(gamma, beta, mean, var):
        g = gamma.astype(np.float32)
        inv = g / np.sqrt(var.astype(np.float32) + np.float32(BN_EPS))
        bias = beta.astype(np.float32) - mean.astype(np.float32) * inv
        return inv.astype(np.float32), bias.astype(np.float32)

    sc_in_v, bi_in_v = fold(inputs["bn_in_gamma"], inputs["bn_in_beta"],
                            inputs["bn_in_mean"], inputs["bn_in_var"])
    sc_mid_v, bi_mid_v = fold(inputs["bn_mid_gamma"], inputs["bn_mid_beta"],
                              inputs["bn_mid_mean"], inputs["bn_mid_var"])
    sc_out_v, bi_out_v = fold(inputs["bn_out_gamma"], inputs["bn_out_beta"],
                              inputs["bn_out_mean"], inputs["bn_out_var"])

    tab = np.zeros((128, 46), dtype=np.float32)
    tab[:, 0:4] = sc_in_v.reshape(4, 128).T          # [p, b]
    tab[:, 4:8] = bi_in_v.reshape(4, 128).T
    tab[:, 8:24] = sc_mid_v.reshape(D * 4, 128).T    # [p, l*b]
    tab[:, 24:40] = bi_mid_v.reshape(D * 4, 128).T
    tab[:, 40:42] = sc_out_v[:256].reshape(2, 128).T
    tab[:, 42:44] = bi_out_v[:256].reshape(2, 128).T
    tab[0, 44] = sc_out_v[256]
    tab[0, 45] = bi_out_v[256]

    shared = {
        "w_in_a": w_in_a, "w_in_b": w_in_b, "w_mid": w_mid_t,
        "w_out_a": w_out_a, "w_out_b": w_out_b, "sbtab": tab,
    }

    in_maps = []
    for b in range(B):
        x_main = np.ascontiguousarray(noisy[b, :256, :])         # [256, 2000]
        xp = np.zeros(TP, dtype=np.float32)
        xp[PAD:PAD + T] = noisy[b, 256, :]
        xk9 = np.zeros((K, TP), dtype=np.float32)
        for k in range(K):
            xk9[k, :TP - k] = xp[k:]
        in_maps.append(dict(shared, x_main=x_main, xk9=xk9))
    return in_maps


def run(inputs, **run_kwargs):
    nc = build_nc()
    in_maps = prep_inputs(inputs)
    res = run_bass_kernel_spmd(nc, in_maps, core_ids=list(range(B)), **run_kwargs)
    out = np.stack([r["y"] for r in res.results]).astype(np.float32)
    return out, res


def kernel(**inputs) -> np.ndarray:
    out, _ = run(inputs)
    return out
